# revision 14
# baseline (speedup 1.0000x reference)
"""Trainium2 Bass kernel for nn_LocalRouter (sparse_attention).

Computation (reference semantics):
  local:  h_w = silu(mu_n @ Wm1_top + mu_{n-w} @ Wm1_bot + bm1), w=1..4
          local = mean_w(h_w) @ Wm2 + bm2
  global: scores = (mu @ Wq) @ (mu @ Wk)^T / sqrt(D), causal; top-8 -> softmax
          global = probs @ mu @ Wv + bv        (rows of probs sum to 1)
  out = concat([local, global]) @ Wo + bo

Algebraic refactors (host-side weight fusion, exact in fp32):
  scores = (mu @ Wqks) @ mu^T, Wqks = Wq @ Wk^T / sqrt(D)
  out = hbar @ Wmo + gsum @ Wvo + bconst
      hbar = sum_w silu(...), Wmo = (Wm2 @ Wo_top)/4, Wvo = Wv @ Wo_bot,
      gsum = sum_k p_k mu[idx_k], bconst = bo + bm2 @ Wo_top + bv @ Wo_bot

Precision: scores need ~2^-20 accuracy (top-8 boundary flips swap whole mu
rows). Computed as a 3-pass fp16 hi/lo split (1 cyc/row on the PE instead of
fp32's 4):
  mu = mh + ml_s/256,  qh = qhh + qhl          (fp16 pairs; lo scaled into
  Wqks = Wh + Wl_s/256                          fp16-normal range)
  qh     = mh@Wh + ml_s@(Wh/256) + (mh/256)@Wl_s      [per-chunk fp32 PSUM]
  scores = qhh@mh + qhl@mh + (qhh/256)@ml_s
Emulated end-to-end: 0/16384 top-8 flips, rel err 3.5e-4 (gate 2e-2).
Everything else runs fp16 (local branch, gather payload, projections).

Top-8 indices are rewrapped for dma_gather on-chip with tiny selection
matmuls (iw[p, c*8+pb] = i8[16*pb + p%16, c]) instead of a DRAM roundtrip.

Sharding: core c -> batch b=c//2, half h=c%2 owns query tiles {t: t%2==h}
(interleaved for causal load balance). Key range per slot s is 256*(s+1)
(h-independent; the h-dependent diagonal lives in the trimask data).
"""

import math
import numpy as np

B, N, D = 4, 4096, 512
WIN, TOPK = 4, 8
P = 128
NCORES = 8
NSLOT = 16            # query tiles owned per core
NEG = -1.0e30
ASC = 256.0           # hi/lo split scale (2^8)

_cache = {}


def _build_program():
    if "nc" in _cache:
        return _cache["nc"]
    from contextlib import ExitStack
    import concourse.bass as bass
    import concourse.tile as tile
    import concourse.mybir as mybir
    from concourse import bacc
    from concourse.masks import make_identity

    dt = mybir.dt
    AF = mybir.ActivationFunctionType
    OP = mybir.AluOpType

    nc = bacc.Bacc(
        "TRN2",
        target_bir_lowering=False,
        debug=False,
        enable_asserts=False,
        num_devices=NCORES,
    )

    f32, f16 = dt.float32, dt.float16
    # ---- DRAM I/O (per-core data; program identical on all cores) ----
    muT_h = nc.dram_tensor("muT_h", [4, P, 4 + N], f16, kind="ExternalInput").ap()
    muT_ls = nc.dram_tensor("muT_ls", [4, P, N], f16, kind="ExternalInput").ap()
    muloc_h = nc.dram_tensor("muloc_h", [4, P, NSLOT * 132], f16,
                             kind="ExternalInput").ap()
    muloc_ls = nc.dram_tensor("muloc_ls", [4, P, NSLOT * 132], f16,
                              kind="ExternalInput").ap()
    mukeys = nc.dram_tensor("mukeys", [N, D], f16, kind="ExternalInput").ap()
    whh = nc.dram_tensor("whh", [4, P, D], f16, kind="ExternalInput").ap()
    whd = nc.dram_tensor("whd", [4, P, D], f16, kind="ExternalInput").ap()
    wls = nc.dram_tensor("wls", [4, P, D], f16, kind="ExternalInput").ap()
    wm1t = nc.dram_tensor("wm1t", [4, P, D], f16, kind="ExternalInput").ap()
    wm1b = nc.dram_tensor("wm1b", [4, P, D], f16, kind="ExternalInput").ap()
    wmo = nc.dram_tensor("wmo", [4, P, D], f16, kind="ExternalInput").ap()
    wvo = nc.dram_tensor("wvo", [4, P, D], f16, kind="ExternalInput").ap()
    trimask = nc.dram_tensor("trimask", [P, 256], f32, kind="ExternalInput").ap()
    bm1t = nc.dram_tensor("bm1t", [P, 4], f32, kind="ExternalInput").ap()
    bconst = nc.dram_tensor("bconst", [P, 4], f32, kind="ExternalInput").ap()
    e8sel = nc.dram_tensor("e8sel", [P, P], f32, kind="ExternalInput").ap()
    repm = nc.dram_tensor("repm", [16, P], f32, kind="ExternalInput").ap()
    outT = nc.dram_tensor("outT", [4, P, NSLOT * P], f32, kind="ExternalOutput").ap()

    with tile.TileContext(nc) as tc, ExitStack() as ctx:
        consts = ctx.enter_context(tc.tile_pool(name="consts", bufs=1))
        qh_pool = ctx.enter_context(tc.tile_pool(name="qh", bufs=2))
        strip_pool = ctx.enter_context(tc.tile_pool(name="strip", bufs=2))
        top_pool = ctx.enter_context(tc.tile_pool(name="top", bufs=2))
        g_pool = ctx.enter_context(tc.tile_pool(name="gather", bufs=2))
        acc_pool = ctx.enter_context(tc.tile_pool(name="acc", bufs=3))
        gt_pool = ctx.enter_context(tc.tile_pool(name="globT", bufs=2))
        loc_pool = ctx.enter_context(tc.tile_pool(name="loc", bufs=2))
        loc1_pool = ctx.enter_context(tc.tile_pool(name="loc1", bufs=1))
        hbar_pool = ctx.enter_context(tc.tile_pool(name="hbar", bufs=1))
        out_pool = ctx.enter_context(tc.tile_pool(name="outstage", bufs=1))

        ps_score = ctx.enter_context(tc.tile_pool(name="ps_score", bufs=3, space="PSUM"))
        ps_qo = ctx.enter_context(tc.tile_pool(name="ps_qo", bufs=2, space="PSUM"))
        ps_small = ctx.enter_context(tc.tile_pool(name="ps_small", bufs=1, space="PSUM"))
        ps_ab = ctx.enter_context(tc.tile_pool(name="ps_ab", bufs=2, space="PSUM"))

        # ---- resident constants ----
        muT_h_sb = consts.tile([P, 4, 4 + N], f16)
        muT_ls_sb = consts.tile([P, 4, N], f16)
        for di in range(4):
            nc.sync.dma_start(muT_h_sb[:, di, :], muT_h[di])
            nc.sync.dma_start(muT_ls_sb[:, di, :], muT_ls[di])
        whh_sb = consts.tile([P, 4, D], f16)
        whd_sb = consts.tile([P, 4, D], f16)
        wls_sb = consts.tile([P, 4, D], f16)
        wm1t_sb = consts.tile([P, 4, D], f16)
        wm1b_sb = consts.tile([P, 4, D], f16)
        wmo_sb = consts.tile([P, 4, D], f16)
        wvo_sb = consts.tile([P, 4, D], f16)
        for sb, dr in ((whh_sb, whh), (whd_sb, whd), (wls_sb, wls),
                       (wm1t_sb, wm1t), (wm1b_sb, wm1b), (wmo_sb, wmo),
                       (wvo_sb, wvo)):
            for di in range(4):
                nc.sync.dma_start(sb[:, di, :], dr[di])
        trimask_sb = consts.tile([P, 256], f32)
        nc.sync.dma_start(trimask_sb[:], trimask[:])
        bm1t_sb = consts.tile([P, 4], f32)
        nc.sync.dma_start(bm1t_sb[:], bm1t[:])
        bconst_sb = consts.tile([P, 4], f32)
        nc.sync.dma_start(bconst_sb[:], bconst[:])
        e8_sb = consts.tile([P, P], f32)
        nc.sync.dma_start(e8_sb[:], e8sel[:])
        rep_sb = consts.tile([16, P], f32)
        nc.sync.dma_start(rep_sb[:], repm[:])
        ident = consts.tile([P, P], f16)
        make_identity(nc, ident[:])
        hbar = hbar_pool.tile([P, 4, NSLOT * P], f16)

        def emit_front(s, mlh, mlls):
            """Stage F: qh (3-pass) + scores strip for slot s. Returns strip."""
            KR = 256 * (s + 1)
            ti = s % 4

            # qh for this slot's 128 queries: psum [do_part, 4 do_tile, q]
            mh_d = qh_pool.tile([P, 4, P], f16, tag="mh_d")
            nc.scalar.activation(mh_d[:], mlh[:, :, ti, 4:132], AF.Identity,
                                 scale=1.0 / ASC)
            qh_ps = ps_qo.tile([P, 4, P], f32, tag="qo")
            for do in range(4):
                for di in range(4):
                    nc.tensor.matmul(
                        qh_ps[:, do, :], whh_sb[:, di, do * P:(do + 1) * P],
                        mlh[:, di, ti, 4:132], start=(di == 0), stop=False)
                for di in range(4):
                    nc.tensor.matmul(
                        qh_ps[:, do, :], whd_sb[:, di, do * P:(do + 1) * P],
                        mlls[:, di, ti, 4:132], start=False, stop=False)
                for di in range(4):
                    nc.tensor.matmul(
                        qh_ps[:, do, :], wls_sb[:, di, do * P:(do + 1) * P],
                        mh_d[:, di, :], start=False, stop=(di == 3))
            qhh = qh_pool.tile([P, 4, P], f16, tag="qhh")
            nc.scalar.copy(qhh[:], qh_ps[:])
            qhl = qh_pool.tile([P, 4, P], f16, tag="qhl")
            nc.vector.tensor_tensor(qhl[:], qh_ps[:], qhh[:], op=OP.subtract)
            qhhd = qh_pool.tile([P, 4, P], f16, tag="qhhd")
            nc.scalar.activation(qhhd[:], qhh[:], AF.Identity, scale=1.0 / ASC)

            # scores strip [128 q, KR keys] fp32 via 3 fp16 passes
            strip = strip_pool.tile([P, N], f32, tag="strip")
            nchunks = (KR + 511) // 512
            for c in range(nchunks):
                k0 = c * 512
                csz = min(512, KR - k0)
                sps = ps_score.tile([P, 512], f32, tag="sps")
                for di in range(4):
                    nc.tensor.matmul(
                        sps[:, :csz], qhh[:, di, :],
                        muT_h_sb[:, di, 4 + k0:4 + k0 + csz],
                        start=(di == 0), stop=False)
                for di in range(4):
                    nc.tensor.matmul(
                        sps[:, :csz], qhhd[:, di, :],
                        muT_ls_sb[:, di, k0:k0 + csz],
                        start=False, stop=False)
                for di in range(4):
                    nc.tensor.matmul(
                        sps[:, :csz], qhl[:, di, :],
                        muT_h_sb[:, di, 4 + k0:4 + k0 + csz],
                        start=False, stop=(di == 3))
                nc.scalar.copy(strip[:, k0:k0 + csz], sps[:, :csz])
            return strip

        def emit_topk_gather(s, strip):
            """Stage TW (runs one slot behind F): mask + top8 + softmax +
            index wrap + gather + weighted sum. Returns acc."""
            KR = 256 * (s + 1)
            # causal mask over the last 256 keys (diagonal + padding)
            nc.vector.tensor_tensor(
                strip[:, KR - 256:KR], strip[:, KR - 256:KR], trimask_sb[:],
                op=OP.add)
            v8 = top_pool.tile([P, TOPK], f32, tag="v8")
            nc.vector.max(out=v8[:], in_=strip[:, :KR])
            i8 = top_pool.tile([P, TOPK], dt.uint32, tag="i8")
            nc.vector.max_index(out=i8[:], in_max=v8[:], in_values=strip[:, :KR])
            nmax = top_pool.tile([P, 1], f32, tag="nmax")
            nc.vector.tensor_scalar_mul(nmax[:], v8[:, 0:1], -1.0)
            e8v = top_pool.tile([P, TOPK], f32, tag="e8v")
            zsum = top_pool.tile([P, 1], f32, tag="zsum")
            nc.scalar.activation(e8v[:], v8[:], AF.Exp, bias=nmax[:],
                                 accum_out=zsum[:])
            zr = top_pool.tile([P, 1], f32, tag="zr")
            nc.vector.reciprocal(zr[:], zsum[:])
            p8 = top_pool.tile([P, TOPK], f32, tag="p8")
            nc.vector.tensor_scalar_mul(p8[:], e8v[:], zr[:])

            # wrap indices for dma_gather: iw[p, c*8+pb] = i8[16*pb+p%16, c]
            # uint32 -> fp32 on GpSimd (the DVE CAST path costs ~4.8us here)
            i8f = top_pool.tile([P, TOPK], f32, tag="i8f")
            nc.gpsimd.tensor_copy(i8f[:], i8[:])
            iw1 = ps_small.tile([16, 64], f32, tag="small")
            iw1v = iw1[:].rearrange("p (c b) -> p b c", b=8)
            for pb in range(8):
                nc.tensor.matmul(iw1v[:, pb, :], e8_sb[:, 16 * pb:16 * pb + 16],
                                 i8f[:], start=True, stop=True,
                                 skip_group_check=True)
            iw1s = top_pool.tile([16, 64], f32, tag="iw1s")
            nc.scalar.copy(iw1s[:], iw1[:])
            iw2 = ps_small.tile([P, 64], f32, tag="small")
            nc.tensor.matmul(iw2[:], rep_sb[:], iw1s[:], start=True, stop=True)
            iw = top_pool.tile([P, 64], dt.int16, tag="iw")
            nc.vector.tensor_copy(iw[:], iw2[:])

            # gather the 8 mu rows per query (fp16, 1KB each) from DRAM
            g = g_pool.tile([P, TOPK, D], f16, tag="g")
            nc.gpsimd.dma_gather(g[:], mukeys[:], iw[:], num_idxs=TOPK * P,
                                 num_idxs_reg=TOPK * P, elem_size=D)
            return g, p8

        def emit_wsum(g, p8):
            """Stage W (two slots behind F): acc = sum_k p8[:,k] * g[:,k,:].
            Per-tap scaling runs on ACT (activation scale is per-partition);
            DVE only does the 7-add tree."""
            for k in range(TOPK):
                nc.scalar.activation(g[:, k, :], g[:, k, :], AF.Identity,
                                     scale=p8[:, k:k + 1])
            for k in range(0, TOPK, 2):
                nc.vector.tensor_tensor(g[:, k, :], g[:, k, :], g[:, k + 1, :],
                                        op=OP.add)
            nc.vector.tensor_tensor(g[:, 0, :], g[:, 0, :], g[:, 2, :], op=OP.add)
            acc = acc_pool.tile([P, D], f16, tag="acc")
            nc.vector.tensor_tensor(acc[:], g[:, 0, :], g[:, 4, :], op=OP.add)
            nc.vector.tensor_tensor(acc[:], acc[:], g[:, 6, :], op=OP.add)
            return acc

        def emit_transpose(s, acc, globalT):
            """Stage X (two slots behind F): acc -> globalT[:, :, ti*128...]."""
            ti = s % 4
            for j in range(4):
                tp = ps_small.tile([P, P], f16, tag="small")
                nc.tensor.transpose(tp[:], acc[:, j * P:(j + 1) * P], ident[:])
                nc.scalar.copy(globalT[:, j, ti * P:(ti + 1) * P], tp[:])

        def emit_local(grp, mlh):
            """Local branch for own-tile group grp (4 tiles, 512 rows):
            hbar[:, dh, grp*512:...] = sum_w silu(A + B_shift(w) + bm1)."""
            r0 = grp * 512
            for dh in range(4):
                a_ps = ps_ab.tile([P, 512], f32, tag="ab")
                for di in range(4):
                    nc.tensor.matmul(
                        a_ps[:], wm1t_sb[:, di, dh * P:(dh + 1) * P],
                        mlh[:, di, :, 4:132], start=(di == 0), stop=(di == 3))
                aP = loc_pool.tile([P, 512], f16, tag="aP")
                nc.scalar.activation(aP[:], a_ps[:], AF.Identity,
                                     bias=bm1t_sb[:, dh:dh + 1])
                bb = loc_pool.tile([P, 4, 132], f16, tag="bb")
                for half in range(2):
                    b_ps = ps_ab.tile([P, 2, 132], f32, tag="ab")
                    for di in range(4):
                        nc.tensor.matmul(
                            b_ps[:], wm1b_sb[:, di, dh * P:(dh + 1) * P],
                            mlh[:, di, 2 * half:2 * half + 2, :],
                            start=(di == 0), stop=(di == 3))
                    nc.scalar.copy(bb[:, 2 * half:2 * half + 2, :], b_ps[:])
                sil = loc1_pool.tile([P, 4, 512], f16, tag="sil")
                for w in range(1, WIN + 1):
                    x = loc_pool.tile([P, 512], f16, tag="xw")
                    nc.vector.tensor_tensor(
                        x[:], aP[:], bb[:, :, 4 - w:132 - w], op=OP.add)
                    nc.scalar.activation(sil[:, w - 1, :], x[:], AF.Silu)
                t1 = loc_pool.tile([P, 512], f16, tag="t1")
                nc.vector.tensor_tensor(t1[:], sil[:, 0, :], sil[:, 1, :],
                                        op=OP.add)
                nc.vector.tensor_tensor(t1[:], t1[:], sil[:, 2, :], op=OP.add)
                nc.vector.tensor_tensor(hbar[:, dh, r0:r0 + 512], t1[:],
                                        sil[:, 3, :], op=OP.add)

        def emit_outproj(grp, globalT):
            r0 = grp * 512
            for do in range(4):
                o_ps = ps_qo.tile([P, 512], f32, tag="qo")
                for dm in range(4):
                    nc.tensor.matmul(
                        o_ps[:], wmo_sb[:, dm, do * P:(do + 1) * P],
                        hbar[:, dm, r0:r0 + 512], start=(dm == 0), stop=False)
                for dm in range(4):
                    nc.tensor.matmul(
                        o_ps[:], wvo_sb[:, dm, do * P:(do + 1) * P],
                        globalT[:, dm, :], start=False, stop=(dm == 3))
                ost = out_pool.tile([P, 512], f32, tag="ost")
                nc.scalar.activation(ost[:], o_ps[:], AF.Identity,
                                     bias=bconst_sb[:, do:do + 1])
                nc.sync.dma_start(outT[do, :, r0:r0 + 512], ost[:])

        # Software-pipelined emission: per-engine streams are in-order, so
        # stage TW (top8+gather, slot s-1) and stage X (transpose, slot s-2)
        # are emitted between stage F (scores, slot s) blocks — the DVE/GPSIMD
        # tail of a slot then overlaps the PE score burst of the next slots.
        st = {}     # per-slot pipeline state
        grpst = {}  # per-group state (mlh, globalT, mlls)
        for s in range(NSLOT + 3):
            if s < NSLOT:
                grp = s // 4
                if s % 4 == 0:
                    mlh = loc_pool.tile([P, 4, 4, 132], f16, tag="mlh")
                    nc.sync.dma_start(
                        mlh[:], muloc_h[:, :, grp * 528:(grp + 1) * 528]
                        .rearrange("a p (t c) -> p a t c", c=132))
                    mlls = loc1_pool.tile([P, 4, 4, 132], f16, tag="mlls")
                    nc.sync.dma_start(
                        mlls[:], muloc_ls[:, :, grp * 528:(grp + 1) * 528]
                        .rearrange("a p (t c) -> p a t c", c=132))
                    globalT = gt_pool.tile([P, 4, 512], f16, tag="globalT")
                    grpst[grp] = (mlh, globalT, mlls)
                mlh, globalT, mlls = grpst[grp]
                st[s] = {"strip": emit_front(s, mlh, mlls)}
            if 1 <= s <= NSLOT:
                st[s - 1]["gp"] = emit_topk_gather(s - 1, st[s - 1]["strip"])
            if 2 <= s <= NSLOT + 1:
                st[s - 2]["acc"] = emit_wsum(*st[s - 2]["gp"])
            if s >= 3:
                sx = s - 3
                emit_transpose(sx, st[sx]["acc"], grpst[sx // 4][1])
                del st[sx]
                if sx % 4 == 3:
                    g = sx // 4
                    emit_local(g, grpst[g][0])
                    emit_outproj(g, grpst[g][1])
                    del grpst[g]

    nc.compile()
    _cache["nc"] = nc
    return nc


def _prep_core_inputs(c, mh_all, mls_all, consts):
    """Host-side sharding/layout for core c (fp16 hi + scaled-lo mu parts)."""
    f16 = np.float16
    b, h = c // 2, c % 2
    mh = mh_all[b]                                # [N, D] f16
    mls = mls_all[b]                              # [N, D] f16
    muT_h = np.zeros((D, 4 + N), f16)
    muT_h[:, 4:] = mh.T
    muT_ls = np.ascontiguousarray(mls.T)
    t_own = list(range(h, 32, 2))

    def strips(m):
        out = np.zeros((NSLOT * 132, D), f16)
        for i, t in enumerate(t_own):
            lo = 128 * t - 4
            src_lo = max(lo, 0)
            out[i * 132 + (src_lo - lo):(i + 1) * 132] = m[src_lo:128 * t + 128]
        return np.ascontiguousarray(out.T)        # [D, NSLOT*132]

    tm = np.zeros((P, 256), np.float32)
    j = np.arange(128)[None, :]
    p = np.arange(128)[:, None]
    if h == 0:
        tm[:, :128] = np.where(j <= p, 0.0, NEG)
        tm[:, 128:] = NEG
    else:
        tm[:, 128:] = np.where(j <= p, 0.0, NEG)
    return dict(
        muT_h=muT_h.reshape(4, P, 4 + N),
        muT_ls=muT_ls.reshape(4, P, N),
        muloc_h=strips(mh).reshape(4, P, NSLOT * 132),
        muloc_ls=strips(mls).reshape(4, P, NSLOT * 132),
        mukeys=mh,
        trimask=tm,
        **consts,
    )


def prep_in_maps(inputs):
    f32, f16 = np.float32, np.float16
    mu = np.asarray(inputs["mu"], f32)
    Wq = np.asarray(inputs["Wq"], f32)
    bq = np.asarray(inputs["bq"], f32)
    Wk = np.asarray(inputs["Wk"], f32)
    Wv = np.asarray(inputs["Wv"], f32)
    bv = np.asarray(inputs["bv"], f32)
    Wm1 = np.asarray(inputs["Wm1"], f32)
    bm1 = np.asarray(inputs["bm1"], f32)
    Wm2 = np.asarray(inputs["Wm2"], f32)
    bm2 = np.asarray(inputs["bm2"], f32)
    Wo = np.asarray(inputs["Wo"], f32)
    bo = np.asarray(inputs["bo"], f32)
    assert not bq.any(), "bq != 0 unsupported (adds a per-key score term)"

    Wqks = (Wq @ Wk.T / math.sqrt(D)).astype(f32)
    Wmo = ((Wm2 @ Wo[:D]) / WIN).astype(f32)
    Wvo = (Wv @ Wo[D:]).astype(f32)
    bconst = (bo + bm2 @ Wo[:D] + bv @ Wo[D:]).astype(f32)

    Wh = Wqks.astype(f16)
    Wl_s = ((Wqks - Wh.astype(f32)) * ASC).astype(f16)
    Wh_d = (Wh.astype(f32) / ASC).astype(f16)
    consts = dict(
        whh=Wh.reshape(4, P, D),
        whd=Wh_d.reshape(4, P, D),
        wls=Wl_s.reshape(4, P, D),
        wm1t=Wm1[:D].astype(f16).reshape(4, P, D),
        wm1b=Wm1[D:].astype(f16).reshape(4, P, D),
        wmo=Wmo.astype(f16).reshape(4, P, D),
        wvo=Wvo.astype(f16).reshape(4, P, D),
        bm1t=np.ascontiguousarray(bm1.reshape(4, P).T),
        bconst=np.ascontiguousarray(bconst.reshape(4, P).T),
        e8sel=np.eye(P, dtype=f32),
        repm=np.ascontiguousarray(np.tile(np.eye(16, dtype=f32), (1, 8))),
    )
    mh_all = mu.astype(f16)                                   # [B, N, D]
    mls_all = ((mu - mh_all.astype(f32)) * ASC).astype(f16)
    return [_prep_core_inputs(c, mh_all, mls_all, consts)
            for c in range(NCORES)]


def assemble(core_outs):
    """core_outs: list of outT arrays [4, P, 2048] per core -> full [B, N, D]."""
    out = np.empty((B, N, D), np.float32)
    for c in range(NCORES):
        b, h = c // 2, c % 2
        oT = np.asarray(core_outs[c])
        oc = np.ascontiguousarray(oT.reshape(D, NSLOT * P).T)  # [2048, D]
        for s, t in enumerate(range(h, 32, 2)):
            out[b, 128 * t:128 * t + 128] = oc[128 * s:128 * s + 128]
    return out


def kernel(**inputs):
    nc = _build_program()
    in_maps = prep_in_maps(inputs)

    import os
    from concourse.bass_utils import run_bass_kernel_spmd
    trace = bool(int(os.environ.get("LR_TRACE", "0")))
    res = run_bass_kernel_spmd(nc, in_maps, core_ids=list(range(NCORES)),
                               trace=trace)
    _cache["last_results"] = res
    return assemble([res.results[c]["outT"] for c in range(NCORES)])
